# revision 26
# baseline (speedup 1.0000x reference)
"""MultiHeadDiffAttention kernel for 8 trn2 NeuronCores — v3.

Sharding: tensor-parallel over heads (H=8, one head per core), fp16 on
device.  Per core (head h), per batch b:
  qT/kT = Wq_h @ x.T  [128, 2048], v = x @ Wv_h.T  [2048 tok, 128 dh]
  per k-chunk: scoresT via row-packed (c=64, concurrent) PE matmuls with
  a one-chunk lookahead, one [128, 2x512] exp on ScalarE (scores ~
  N(0,1): no max subtraction), uT += v.T @ exp on PE, exp accumulated
  into esum on the Vector engine (ping-pong).

v3 restructure vs v2 (297us):
  * The whole kernel is driven by one background FIFO of ~2300-cycle PE
    thunks (projection half-groups / v pairs / phase-3 pieces) pumped
    with a cycle-debt governor, so the PE queue never drains: b1's k/v
    projections are pulled forward into b0's attention window and b1's
    window runs ACT-paced with filler matmuls topping up the PE.  This
    keeps the Tensor engine's HAM clock at max pstate (the v2 profile
    showed the whole batch-1 window running 512-row matmuls at 1.2GHz).
  * Tokens are redistributed head-sharded -> token-sharded with FOUR
    AllToAlls (2 per batch, 128-token stripes per core per half): core c
    owns tokens [c*128,(c+1)*128) and [1024+c*128, ...) of EACH batch.
    Each a2a [8, 128, 128] fp16 fires 1.5 q-blocks before the batch
    ends, so only the last quarter-batch a2a (~264KB) plus one 128-token
    phase-3 is exposed (v2 exposed a 528KB a2a + 256-token phase 3,
    ~70us).
  * Per-head RMS partials are no longer shipped: phase 3 squares the
    gathered oTh and reduces with ones-matmuls (PSUM-accumulated over
    heads) into a per-token COLUMN, turns it into 1/rms via Ln/Exp on
    ScalarE (exp and ln share one ACT table -> no table swaps), and
    folds the scale into the Wo PSUM eviction as a per-partition
    tensor_scalar multiply.  norm_w and (1-dw) stay folded into Wo.
  * Softmax-tail reciprocal rows are broadcast across partitions with
    gpsimd.partition_broadcast and the u*r muls run on GpSimd (idle in
    v2), keeping DVE under the exp pacing.
Host concatenates the 8x4 slices per batch and adds (1-dw)*bo.
"""

import os
import sys

import numpy as np

if "/opt/trn_rl_repo" not in sys.path:
    sys.path.insert(0, "/opt/trn_rl_repo")

B, S, E, H = 2, 2048, 1024, 8
DH = E // H          # 128
F = DH // 2          # 64
P = 128              # partitions
NCORES = 8
TPC = 512            # output tokens per core (4 blocks of 128)
HB = 128             # tokens per a2a block (1/4 q-block-pair)
EC = E // P          # 8 e-chunks
KC = S // P          # 16 k-chunks per batch
QBS = 512            # q-block size
QB = S // QBS        # 4 q-blocks per batch
EPS = float(np.finfo(np.float32).eps)

LAST_RESULTS = None  # BassKernelResults of the most recent run (test.py reads this)

_NC_CACHE: dict = {}


def _build(dw: float):
    import concourse.bass as bass
    import concourse.mybir as mybir
    import concourse.tile as tile
    from concourse import bacc

    dt = mybir.dt
    AF = mybir.ActivationFunctionType

    nc = bacc.Bacc("TRN2", target_bir_lowering=False, debug=False, num_devices=NCORES)

    xT_d = nc.dram_tensor("xT", [B, E, S], dt.float16, kind="ExternalInput")
    wqT_d = nc.dram_tensor("wqT", [P, E], dt.float16, kind="ExternalInput")
    wkT_d = nc.dram_tensor("wkT", [P, E], dt.float16, kind="ExternalInput")
    wvT_d = nc.dram_tensor("wvT", [P, E], dt.float16, kind="ExternalInput")
    woT_d = nc.dram_tensor("woT", [P, EC * E], dt.float16, kind="ExternalInput")
    out_d = nc.dram_tensor("out", [TPC, E], dt.float16, kind="ExternalOutput")

    with tile.TileContext(nc) as tc:
        with (
            tc.tile_pool(name="consts", bufs=1) as consts,
            tc.tile_pool(name="xt", bufs=1) as xtp,
            tc.tile_pool(name="qk", bufs=1) as qkp,
            tc.tile_pool(name="vp", bufs=1) as vp,
            tc.tile_pool(name="expp", bufs=4) as expp,
            tc.tile_pool(name="esum", bufs=2) as esump,
            tc.tile_pool(name="osb", bufs=2) as osb,
            tc.tile_pool(name="small", bufs=2) as small,
            tc.tile_pool(name="mid", bufs=2) as mid,
            tc.tile_pool(name="p3", bufs=2) as p3,
            tc.tile_pool(name="dram", bufs=1, space="DRAM") as dram,
            tc.tile_pool(name="psA", bufs=2, space="PSUM") as psA,
            tc.tile_pool(name="psU", bufs=2, space="PSUM") as psU,
            tc.tile_pool(name="psS", bufs=2, space="PSUM") as psS,
        ):
            eps_t = consts.tile([P, 1], dt.float32, tag="eps")
            nc.vector.memset(eps_t, EPS)
            ones_col = consts.tile([P, 32], dt.float16, tag="ones_col")
            nc.vector.memset(ones_col, 1.0)

            # ACT warm-up: Exp + Ln share the natural_log_exp table, so
            # warming both here pins ONE table for the whole kernel.
            warm = consts.tile([P, 4], dt.float32, tag="warm")
            nc.vector.memset(warm, 1.0)
            warm2 = consts.tile([P, 4], dt.float32, tag="warm2")
            nc.scalar.activation(warm2, warm, AF.Exp)

            wq_sb = consts.tile([P, EC, DH], dt.float16, tag="wq")
            wk_sb = consts.tile([P, EC, DH], dt.float16, tag="wk")
            wv_sb = consts.tile([P, EC, DH], dt.float16, tag="wv")
            nc.sync.dma_start(
                out=wk_sb, in_=wkT_d.rearrange("p (c d) -> p c d", c=EC)
            )

            # Per (batch, half) AllToAll buffers: block d = [oT rows 0..127]
            # x [128 tokens for dest core d].
            a2a_in = [
                [dram.tile([NCORES, DH, HB], dt.float16, tag=f"a2a_in{b}{hf}",
                           name=f"a2a_in{b}{hf}") for hf in range(2)]
                for b in range(B)
            ]
            a2a_out = [
                [dram.tile([NCORES, DH, HB], dt.float16, tag=f"a2a_out{b}{hf}",
                           name=f"a2a_out{b}{hf}") for hf in range(2)]
                for b in range(B)
            ]

            # ---------- background work queue (cycle-debt paced) ----------
            # entries are (cost_cycles, due_chunk, thunk).  due_chunk is the
            # absolute k-chunk index by which the thunk MUST have been
            # emitted (its first reader's emission follows right after) —
            # the debt governor paces early emission for PE-feed, the due
            # date is the correctness backstop.
            bg: list = []
            state = {"debt": 0.0, "filler": False, "chunk": 0}

            def ps1():
                t = psS.tile([P, QBS], dt.float32, tag="ps1")
                return t

            def filler():
                # keeps the PE pipeline occupied (HAM clock at max pstate):
                # bare weight loads touch no PSUM, so they can never create
                # cross-engine waits.  Reading the PREVIOUS chunk's exp tile
                # pins them in schedule order (a dep-free instruction gets
                # hoisted to kernel start by the scheduler) without ever
                # stalling the PE queue (that exp is already complete).
                pin = state.get("pin")
                src_t = pin if pin is not None else wk_sb[:, 0, :]
                for _ in range(5):
                    nc.tensor.ldweights(src_t)

            def pump(rate):
                state["debt"] = min(state["debt"] + rate, 9000.0)
                popped = False
                while bg and (bg[0][1] <= state["chunk"]
                              or state["debt"] >= bg[0][0]):
                    cost, due, th = bg.pop(0)
                    state["debt"] -= cost
                    th()
                    popped = True
                state["debt"] = max(state["debt"], -9000.0)
                state["popped"] = popped

            # ---------- projection helpers ----------
            def qk_half(w_sb_, dst, tb, half, xt):
                # one 256-token half-group: 8 matmuls + one eviction cast
                cs = slice(tb * QBS + half * 256, tb * QBS + (half + 1) * 256)
                ps = ps1()
                w = cs.stop - cs.start
                for ec in range(EC):
                    nc.tensor.matmul(
                        ps[:, 0:w],
                        lhsT=w_sb_[:, ec, :],
                        rhs=xt[:, ec, cs],
                        start=(ec == 0),
                        stop=(ec == EC - 1),
                    )
                nc.vector.tensor_copy(dst[:, cs], ps[:, 0:w])

            def v_pair(kt2, v, xt):
                # two k-chunks of v in one PSUM bank + one eviction cast
                ps = ps1()
                for j in range(2):
                    for ec in range(EC):
                        nc.tensor.matmul(
                            ps[:, j * DH : (j + 1) * DH],
                            lhsT=xt[:, ec, (kt2 + j) * P : (kt2 + j + 1) * P],
                            rhs=wv_sb[:, ec, :],
                            start=(ec == 0),
                            stop=(ec == EC - 1),
                        )
                nc.vector.tensor_copy(
                    v[:, kt2 : kt2 + 2, :].rearrange("p a b -> p (a b)"),
                    ps[:, 0 : 2 * DH],
                )

            # ---------- per-batch tiles ----------
            qT = [qkp.tile([P, S], dt.float16, tag=f"qT{b}", name=f"qT{b}")
                  for b in range(B)]
            kT = [qkp.tile([P, S], dt.float16, tag=f"kT{b}", name=f"kT{b}")
                  for b in range(B)]
            v = [vp.tile([P, KC, DH], dt.float16, tag=f"v{b}", name=f"v{b}")
                 for b in range(B)]
            xts = [xtp.tile([P, EC, S], dt.float16, tag=f"xt{b}", name=f"xt{b}")
                   for b in range(B)]

            # ---------- batch 0: x DMA (per-ec for tb0 so proj starts early) --
            xT_v0 = xT_d[0].rearrange("(c p) t -> p c t", p=P)
            for ec in range(EC):
                nc.sync.dma_start(
                    out=xts[0][:, ec : ec + 1, 0:QBS],
                    in_=xT_v0[:, ec : ec + 1, 0:QBS],
                )
            nc.sync.dma_start(
                out=wv_sb, in_=wvT_d.rearrange("p (c d) -> p c d", c=EC)
            )
            nc.sync.dma_start(
                out=wq_sb, in_=wqT_d.rearrange("p (c d) -> p c d", c=EC)
            )
            for tb in range(1, QB):
                nc.sync.dma_start(
                    out=xts[0][:, :, tb * QBS : (tb + 1) * QBS],
                    in_=xT_v0[:, :, tb * QBS : (tb + 1) * QBS],
                )

            # batch-0 projection prologue: what qb0's first k-chunks need
            qk_half(wk_sb, kT[0], 0, 0, xts[0])
            qk_half(wk_sb, kT[0], 0, 1, xts[0])
            v_pair(0, v[0], xts[0])
            qk_half(wq_sb, qT[0], 0, 0, xts[0])
            qk_half(wq_sb, qT[0], 0, 1, xts[0])

            # batch-1 x DMA + wo DMA (run on DMA queues under b0 attention)
            xT_v1 = xT_d[1].rearrange("(c p) t -> p c t", p=P)
            for tb in range(QB):
                nc.sync.dma_start(
                    out=xts[1][:, :, tb * QBS : (tb + 1) * QBS],
                    in_=xT_v1[:, :, tb * QBS : (tb + 1) * QBS],
                )
            wo_sb = consts.tile([P, EC, E], dt.float16, tag="wo")
            nc.sync.dma_start(out=wo_sb, in_=woT_d.rearrange("p (c e) -> p c e", c=EC))

            QKC = 2600.0   # est cycles per qk half-group thunk (incl ldw)
            VPC = 2600.0   # est cycles per v pair thunk

            # b0 qb0-critical thunks: remaining kT halves + v pairs, v pair
            # j due at chunk 2j, k half (tb,h) due at chunk 4tb+2h
            def k_th(b, tb, h):
                # first read: scores(4tb+2h) emitted at top of chunk 4tb+2h-1
                due = 64 * b + max(4 * tb + 2 * h - 2, 0)
                return (QKC, due, lambda: qk_half(wk_sb, kT[b], tb, h, xts[b]))

            def q_th(b, tb, h):
                # first read: scores(0) of q-block tb, emitted right after
                # chunk 16*tb - 1 of the same batch
                due = 64 * b + max(16 * tb - 1, 0)
                return (QKC, due, lambda: qk_half(wq_sb, qT[b], tb, h, xts[b]))

            def v_th(b, j):
                # first read: consume(2j) emitted in chunk 2j+1 before pump
                return (VPC, 64 * b + 2 * j,
                        lambda: v_pair(2 * j, v[b], xts[b]))

            bg.extend([
                v_th(0, 1), k_th(0, 1, 0), v_th(0, 2), k_th(0, 1, 1),
                v_th(0, 3), k_th(0, 2, 0), v_th(0, 4), k_th(0, 2, 1),
                v_th(0, 5), k_th(0, 3, 0), v_th(0, 6), k_th(0, 3, 1),
                v_th(0, 7),
                q_th(0, 1, 0), q_th(0, 1, 1), q_th(0, 2, 0), q_th(0, 2, 1),
                q_th(0, 3, 0), q_th(0, 3, 1),
                # batch-1 k/v pulled forward into b0's qb1-3 window
                k_th(1, 0, 0), k_th(1, 0, 1), v_th(1, 0), v_th(1, 1),
                k_th(1, 1, 0), k_th(1, 1, 1), v_th(1, 2), v_th(1, 3),
                k_th(1, 2, 0), k_th(1, 2, 1), v_th(1, 4), v_th(1, 5),
                k_th(1, 3, 0), k_th(1, 3, 1), v_th(1, 6), v_th(1, 7),
                q_th(1, 0, 0), q_th(1, 0, 1),
            ])

            # full-size dummy AllToAll: absorbs the first-collective setup
            # latency AND any first-large-op ring allocation in the CC
            # runtime, so the real a2as hit a fully warmed path
            cc_warm_in = dram.tile([NCORES, DH, HB], dt.float16, tag="ccwi",
                                   name="ccwi")
            cc_warm_out = dram.tile([NCORES, DH, HB], dt.float16, tag="ccwo",
                                    name="ccwo")
            warmrow = consts.tile([P, NCORES, HB], dt.float16, tag="warmrow")
            nc.vector.memset(warmrow, 0.0)
            nc.sync.dma_start(
                out=cc_warm_in.rearrange("c p t -> p c t"), in_=warmrow
            )
            nc.gpsimd.collective_compute(
                "AllToAll",
                mybir.AluOpType.bypass,
                replica_groups=[list(range(NCORES))],
                ins=[cc_warm_in.opt()],
                outs=[cc_warm_out.opt()],
            )

            def emit_a2a(b, hf):
                nc.gpsimd.collective_compute(
                    "AllToAll",
                    mybir.AluOpType.bypass,
                    replica_groups=[list(range(NCORES))],
                    ins=[a2a_in[b][hf].opt()],
                    outs=[a2a_out[b][hf].opt()],
                )

            # ---------- softmax tails ----------
            def make_tails(b, qb, u1s, u2s, esum_t, fast=False):
                # fast=True runs the u*r muls on DVE (shorter critical chain,
                # used for the final exposed tail); otherwise GpSimd.
                st: dict = {}
                eng = nc.vector if fast else nc.gpsimd
                state["pinE"] = esum_t[:, 0, 0:DH]

                def tail_a():
                    dsum1 = ps1()
                    nc.tensor.matmul(dsum1[0:32, :], lhsT=ones_col,
                                     rhs=esum_t[:, 0, :])
                    dsum2 = ps1()
                    nc.tensor.matmul(dsum2[0:32, :], lhsT=ones_col,
                                     rhs=esum_t[:, 1, :])
                    rrow1f = small.tile([1, QBS], dt.float32, tag="rrowf")
                    rrow2f = small.tile([1, QBS], dt.float32, tag="rrowf")
                    nc.vector.reciprocal_approx_fast(rrow1f, dsum1[0:1, :])
                    nc.vector.reciprocal_approx_fast(rrow2f, dsum2[0:1, :])
                    rrow1 = small.tile([1, QBS], dt.float16, tag="rrow")
                    rrow2 = small.tile([1, QBS], dt.float16, tag="rrow")
                    nc.vector.tensor_copy(rrow1, rrow1f)
                    nc.vector.tensor_scalar_mul(rrow2, rrow2f, -dw)
                    st["rrow"] = (rrow1, rrow2)

                def tail_b():
                    rrow1, rrow2 = st["rrow"]
                    rr = mid.tile([P, 2, QBS], dt.float16, tag="rr")
                    nc.gpsimd.partition_broadcast(rr[:, 0, :], rrow1)
                    nc.gpsimd.partition_broadcast(rr[:, 1, :], rrow2)
                    t1 = mid.tile([P, QBS], dt.float32, tag="t1")
                    eng.tensor_mul(t1, u1s, rr[:, 0, :])
                    t2 = mid.tile([P, QBS], dt.float32, tag="t2")
                    eng.tensor_mul(t2, u2s, rr[:, 1, :])
                    oT = osb.tile([P, QBS], dt.float16, tag="oT")
                    eng.tensor_add(oT, t1, t2)
                    st["oT"] = oT
                    state["pinT"] = oT[:, 0:DH]

                def tail_c():
                    # stage my 4 dest-core stripes of this q-block
                    hf = qb // 2
                    bc = (qb % 2) * 4
                    nc.sync.dma_start(
                        out=a2a_in[b][hf][bc : bc + 4]
                        .rearrange("d p t -> p d t"),
                        in_=st["oT"].rearrange("p (d t) -> p d t", d=4),
                    )

                return tail_a, tail_b, tail_c

            # ---------- phase 3 (per batch-half: my 128 tokens) ----------
            out_v = out_d.rearrange("(q p) e -> q p e", p=P)

            def make_phase3_thunks(b, hf, due=10000):
                st: dict = {}

                def load():
                    st["oTh"] = p3.tile([P, H, HB], dt.float16, tag="oTh",
                                        name="oTh")
                    state["pin3"] = st["oTh"][:, 0, :]
                    nc.sync.dma_start(
                        out=st["oTh"],
                        in_=a2a_out[b][hf].rearrange("h p t -> p h t"),
                    )

                def rms():
                    # per-token 1/rms column: square, ones-matmul over
                    # (dh, heads) into a column, then a DVE-only Newton
                    # rsqrt (quake seed + 2 iterations) so the ACT engine
                    # never swaps away from its exp table
                    sq = osb.tile([P, H, HB], dt.float16, tag="sq")
                    nc.vector.tensor_mul(sq, st["oTh"], st["oTh"])
                    ps = ps1()
                    for h in range(H):
                        nc.tensor.matmul(
                            ps[:, 0:1],
                            lhsT=sq[:, h, :],
                            rhs=ones_col[:, 0:1],
                            start=(h == 0),
                            stop=(h == H - 1),
                        )
                    alu = mybir.AluOpType
                    ms = small.tile([P, 1], dt.float32, tag="ms")
                    nc.vector.tensor_scalar(ms, ps[:, 0:1], 1.0 / E, EPS,
                                            alu.mult, alu.add)
                    msu = ms.bitcast(dt.uint32)
                    half = small.tile([P, 1], dt.float32, tag="half")
                    nc.vector.tensor_scalar(half, ms, 0.5, None, alu.mult)
                    y = small.tile([P, 1], dt.float32, tag="y0")
                    yu = y.bitcast(dt.uint32)
                    nc.vector.tensor_scalar(yu, msu, 1, None,
                                            alu.logical_shift_right)
                    yi = y.bitcast(dt.int32)
                    nc.vector.tensor_scalar(yi, yi, 0x5F3759DF, None,
                                            alu.subtract)
                    nc.vector.tensor_scalar(yi, yi, -1.0, None, alu.mult)
                    t1_ = small.tile([P, 1], dt.float32, tag="nt1")
                    t2_ = small.tile([P, 1], dt.float32, tag="nt2")
                    for _ in range(2):
                        nc.vector.tensor_mul(t1_, y, y)
                        nc.vector.tensor_mul(t2_, t1_, half)
                        nc.vector.tensor_scalar(t2_, t2_, -1.0, 1.5,
                                                alu.mult, alu.add)
                        nc.vector.tensor_mul(y, y, t2_)
                    st["rms"] = y
                    st["out_sb"] = p3.tile([P, E], dt.float16, tag="out_sb",
                                           name="out_sb")

                def wo_chunk(nb):
                    acc = ps1()
                    for fc in range(EC):
                        nc.tensor.matmul(
                            acc,
                            lhsT=st["oTh"][:, fc, :],
                            rhs=wo_sb[:, fc, nb * QBS : (nb + 1) * QBS],
                            start=(fc == 0),
                            stop=(fc == EC - 1),
                        )
                    # fold the per-token rms scale into the eviction
                    nc.vector.tensor_scalar_mul(
                        st["out_sb"][:, nb * QBS : (nb + 1) * QBS], acc,
                        st["rms"],
                    )
                    nc.sync.dma_start(
                        out=out_v[2 * b + hf][:, nb * QBS : (nb + 1) * QBS],
                        in_=st["out_sb"][:, nb * QBS : (nb + 1) * QBS],
                    )

                return [(0.0, due, load), (1700.0, due + 1, rms),
                        (4400.0, due + 2, lambda: wo_chunk(0)),
                        (4400.0, due + 3, lambda: wo_chunk(1))]

            # ---------- attention ----------
            def attention(b, carry=(), qb_hook=None):
                qTb, kTb, vb = qT[b], kT[b], v[b]
                pending: dict = dict(carry)
                last_tails = None

                for qb in range(QB):
                    if qb_hook is not None:
                        qb_hook(qb)
                    qs = slice(qb * QBS, (qb + 1) * QBS)
                    u1 = psU.tile([P, QBS], dt.float32, tag="u")
                    u2 = psU.tile([P, QBS], dt.float32, tag="u")

                    def scores_alloc():
                        s12 = psA.tile([P, 2, QBS], dt.float32, tag="sc")
                        return s12

                    def scores_into(s12, kt):
                        ks = slice(kt * P, (kt + 1) * P)
                        nc.tensor.matmul(s12[:, 0, :], lhsT=kTb[0:F, ks],
                                         rhs=qTb[0:F, qs])
                        nc.tensor.matmul(s12[:, 1, :], lhsT=kTb[F:P, ks],
                                         rhs=qTb[F:P, qs])
                        return s12

                    def scores(kt):
                        return scores_into(scores_alloc(), kt)

                    def consume(kt, ee):
                        nc.tensor.matmul(
                            u1, lhsT=vb[:, kt, :], rhs=ee[:, 0, :],
                            start=(kt == 0), stop=(kt == KC - 1),
                        )
                        nc.tensor.matmul(
                            u2, lhsT=vb[:, kt, :], rhs=ee[:, 1, :],
                            start=(kt == 0), stop=(kt == KC - 1),
                        )

                    if b == 0 and qb == 0:
                        rate = 2100.0
                    elif b == 0:
                        rate = 1250.0
                    else:
                        rate = 1150.0

                    s12 = scores(0)
                    prev = None
                    esum_prev = None
                    for kt in range(KC):
                        if kt + 1 < KC:
                            s12_next = scores_alloc()
                            if state["filler"] and not state.get("popped", True):
                                for _ in range(2):
                                    nc.tensor.matmul(
                                        s12_next[0:32, 0, :], lhsT=ones_col,
                                        rhs=kTb[:, 0:QBS],
                                        start=True, stop=True,
                                    )
                            scores_into(s12_next, kt + 1)
                        else:
                            s12_next = None
                        ee = expp.tile([P, 2, QBS], dt.float16, tag="ee")
                        nc.scalar.activation(ee, s12, AF.Exp, scale=F**-0.5)
                        s12 = s12_next
                        # previous q-block's tail pieces, spread over the
                        # early k-chunks so their serial chains hide under
                        # the exp pipeline
                        if kt in pending:
                            pending.pop(kt)()
                        # consume the PREVIOUS k-chunk (lag-1 pipeline)
                        if prev is not None:
                            consume(*prev)
                        state["pin"] = (prev[1] if prev is not None
                                        else ee)[:, 0, 0:DH]
                        prev = (kt, ee)
                        pump(rate)
                        state["chunk"] += 1
                        # DVE accumulation of exp for the denominators
                        esum_t = esump.tile([P, 2, QBS], dt.float16, tag="es")
                        if esum_prev is None:
                            nc.vector.tensor_copy(esum_t, ee)
                        else:
                            nc.vector.tensor_add(esum_t, esum_prev, ee)
                        esum_prev = esum_t
                    consume(*prev)
                    # evict u to SBUF so the PSUM banks (and the next
                    # q-block's consume) don't wait on the softmax chain
                    u1s = mid.tile([P, QBS], dt.float32, tag="u1s")
                    nc.vector.tensor_copy(u1s, u1)
                    u2s = mid.tile([P, QBS], dt.float32, tag="u2s")
                    nc.vector.tensor_copy(u2s, u2)

                    last = (qb == QB - 1)
                    tails = make_tails(b, qb, u1s, u2s, esum_prev,
                                       fast=(b == 1 and last))
                    if last:
                        last_tails = tails
                    else:
                        pending = {1: tails[0], 3: tails[1], 5: tails[2]}
                        if qb == 1:
                            # half-A a2a right after qb1's tails land
                            pending[9] = lambda: emit_a2a(b, 0)
                return last_tails

            # ---------- main flow ----------
            state["filler"] = False
            b0_tails = attention(0)

            carry = (
                (1, b0_tails[0]),
                (3, b0_tails[1]),
                (5, b0_tails[2]),
                (9, lambda: emit_a2a(0, 1)),
            )
            # make sure every batch-1 projection queued for b0's window has
            # actually been emitted before b1's attention reads qT/kT/v
            while bg:
                pump(9000.0)
            # phase-3 pieces are inserted per-qb so each phase-3 load is
            # emitted strictly after its a2a trigger (and with a full
            # q-block of slack for the collective to land).
            bg.extend([q_th(1, 1, 0), q_th(1, 1, 1)])

            def b1_hook(qb):
                if qb == 1:
                    bg.extend([q_th(1, 2, 0), q_th(1, 2, 1)])
                    bg.extend(make_phase3_thunks(0, 0))
                    bg.extend(make_phase3_thunks(0, 1))
                elif qb == 2:
                    bg.extend([q_th(1, 3, 0), q_th(1, 3, 1)])
                elif qb == 3:
                    # (b1, half A)'s a2a fired at b1-qb2 slot 7
                    bg.extend(make_phase3_thunks(1, 0))

            state["filler"] = True
            b1_tails = attention(1, carry=carry, qb_hook=b1_hook)

            # drain whatever background work is left (b1 half-A phase 3)
            while bg:
                pump(9000.0)

            # final exposed tail: qb3 softmax chain + half-B a2a + phase 3
            def pe_warm(n, pin=None):
                ps = ps1()
                rhs = pin if pin is not None else kT[1][:, 0:QBS]
                for i in range(n):
                    nc.tensor.matmul(
                        ps[0:32, 0 : rhs.shape[-1]], lhsT=ones_col, rhs=rhs,
                        start=(i == 0), stop=(i == n - 1),
                    )

            b1_tails[0]()
            pe_warm(6, state.get("pinE"))
            b1_tails[1]()
            pe_warm(4, state.get("pinT"))
            b1_tails[2]()
            emit_a2a(1, 1)
            tail3 = make_phase3_thunks(1, 1)
            tail3[0][2]()  # oTh load (DMA) queues behind the collective
            pe_warm(40, state.get("pinT"))  # PE ticks across the a2a
            tail3[1][2]()  # rms chain
            pe_warm(6, state.get("pin3"))
            tail3[2][2]()
            tail3[3][2]()

    nc.compile()
    return nc


def _get_nc(dw: float):
    key = round(float(dw), 9)
    if key not in _NC_CACHE:
        _NC_CACHE[key] = _build(float(dw))
    return _NC_CACHE[key]


def _ensure_ntff_shim():
    """bass_utils imports antenv.axon_hooks when trace=True under axon; some
    images lack it.  Provide a ctypes-backed stand-in so tracing (if
    requested via KERNEL_TRACE) doesn't crash the run."""
    import types

    try:
        import antenv

        if hasattr(antenv, "axon_hooks"):
            return
        from trn_agent_boot.trn_boot import _ntff_profile_via_ctypes

        hook = _ntff_profile_via_ctypes("/opt/axon/libaxon_pjrt.so")
        mod = types.ModuleType("antenv.axon_hooks")
        mod.get_axon_ntff_profile_hook = lambda: hook
        mod.set_axon_ntff_profile_hook = lambda h: None
        sys.modules["antenv.axon_hooks"] = mod
        antenv.axon_hooks = mod
    except Exception:
        pass


def kernel(x, Wq, Wk, Wv, norm_w, Wo, bo, diff_weight):
    from concourse.bass_utils import run_bass_kernel_spmd

    global LAST_RESULTS

    if os.environ.get("KERNEL_TRACE"):
        _ensure_ntff_shim()

    f16 = np.float16
    x = np.asarray(x, dtype=np.float32)
    Wq = np.asarray(Wq, dtype=np.float32)
    Wk = np.asarray(Wk, dtype=np.float32)
    Wv = np.asarray(Wv, dtype=np.float32)
    Wo = np.asarray(Wo, dtype=np.float32)
    norm_w = np.asarray(norm_w, dtype=np.float32)
    bo = np.asarray(bo, dtype=np.float32)
    dw = float(np.asarray(diff_weight))

    nc = _get_nc(dw)

    def pack_w(wT):
        # [E, D] -> [P, EC*D]: row p holds the 8 contraction chunks
        # contiguously, so the weight DMA moves 2KB+ lines per partition
        d = wT.shape[1]
        return np.ascontiguousarray(
            wT.reshape(EC, P, d).transpose(1, 0, 2).reshape(P, EC * d)
        ).astype(f16)

    xT = np.ascontiguousarray(x.transpose(0, 2, 1)).astype(f16)  # [B, E, S]
    woT = pack_w((Wo * norm_w.reshape(-1)[None, :] * (1.0 - dw)).T)

    in_maps = []
    for h in range(NCORES):
        rows = slice(h * DH, (h + 1) * DH)
        in_maps.append(
            {
                "xT": xT,
                "wqT": pack_w(np.ascontiguousarray(Wq[rows, :].T)),
                "wkT": pack_w(np.ascontiguousarray(Wk[rows, :].T)),
                "wvT": pack_w(np.ascontiguousarray(Wv[rows, :].T)),
                "woT": woT,
            }
        )

    res = run_bass_kernel_spmd(
        nc,
        in_maps,
        core_ids=list(range(NCORES)),
        trace=bool(os.environ.get("KERNEL_TRACE")),
    )
    LAST_RESULTS = res

    # core c: out rows [(2b+hf)*128 : ...+128] = batch-b tokens
    # [hf*1024 + c*128, +128)
    full = np.empty((B * S, E), dtype=np.float32)
    for c in range(NCORES):
        o = np.asarray(res.results[c]["out"], dtype=np.float32)
        for b in range(B):
            for hf in range(2):
                r0 = (2 * b + hf) * HB
                t0 = b * S + hf * 1024 + c * HB
                full[t0 : t0 + HB] = o[r0 : r0 + HB]
    full = full + (1.0 - dw) * bo[None, :]
    return full.reshape(B, S, E).astype(np.float32)


if __name__ == "__main__":
    rng = np.random.default_rng(0)
    sc = E**-0.5
    ins = {
        "x": rng.standard_normal((B, S, E), dtype=np.float32),
        "Wq": rng.standard_normal((E, E), dtype=np.float32) * sc,
        "Wk": rng.standard_normal((E, E), dtype=np.float32) * sc,
        "Wv": rng.standard_normal((E, E), dtype=np.float32) * sc,
        "norm_w": np.ones((H, DH), dtype=np.float32),
        "Wo": rng.standard_normal((E, E), dtype=np.float32) * sc,
        "bo": np.zeros((E,), dtype=np.float32),
        "diff_weight": np.float32(0.2),
    }
    out = kernel(**ins)
    print("out", out.shape, out.dtype, float(np.abs(out).max()))
